# revision 2
# baseline (speedup 1.0000x reference)
"""GCN 3-layer message-passing kernel for TRN2 (8 NeuronCores, SPMD).

Self-contained: takes FULL inputs, shards internally, runs a Bass/Tile
kernel via run_bass_kernel_spmd on cores 0-7, gathers the full output.

v2 strategy (vs baseline):
  - fp16 tables everywhere: halves gather DMA, AllGather bytes, and
    doubles PE matmul throughput. Validated ~3e-5 rel err on host.
  - Gathers grouped over G=8 target blocks per source chunk: 52
    dma_gather calls per layer instead of 392, amortizing the ~1.6us
    SWDGE fixed cost on the Pool engine.
  - Slot-aligned equalized padding: every (block, chunk) section is
    padded with dummy index-0 entries to its shared slot count, so a
    grouped gather call has no interior negative indices and num_idxs
    is a compile-time immediate (no count registers at all).
  - Per-group PSUM accumulation across the 4 source chunks; bias via
    identity matmul; relu fused in the ACT evacuation.
"""

import os
import sys

sys.path.insert(0, "/opt/trn_rl_repo")

import numpy as np

import concourse.bass as bass  # noqa: F401
import concourse.mybir as mybir
import concourse.tile as tile
from concourse import bacc
from concourse._compat import cdiv
from concourse.bass_utils import run_bass_kernel_spmd

F32 = mybir.dt.float32
F16 = mybir.dt.float16
I16 = mybir.dt.int16
AL = mybir.AluOpType
AF = mybir.ActivationFunctionType

NC = 8
P = 128
G = 8  # target blocks per gather group

LAST_EXEC_NS = None
LAST_SCOPES = None


def _cdiv_np(a, b):
    return (a + b - 1) // b


def _group_cumcount(grp: np.ndarray) -> np.ndarray:
    n = len(grp)
    if n == 0:
        return np.zeros(0, dtype=np.int64)
    is_new = np.ones(n, dtype=bool)
    is_new[1:] = grp[1:] != grp[:-1]
    idx = np.arange(n)
    start = np.maximum.accumulate(np.where(is_new, idx, 0))
    return idx - start


def _preprocess(edge_index: np.ndarray, n_nodes: int, chunk: int):
    N = n_nodes
    S = N // NC
    NB = cdiv(S, P)
    NG = _cdiv_np(NB, G)
    NQ = cdiv(N, chunk)

    src = np.concatenate([edge_index[0], np.arange(N, dtype=np.int64)])
    dst = np.concatenate([edge_index[1], np.arange(N, dtype=np.int64)])
    deg = np.bincount(dst, minlength=N).astype(np.float64)
    dis = 1.0 / np.sqrt(deg)
    norm = (dis[src] * dis[dst]).astype(np.float32)

    core = dst // S
    blk = (dst % S) // P
    tloc = (dst % S) % P
    grp = blk // G
    q = src // chunk
    sloc = src - q * chunk

    counts = np.zeros((NC, NB, NQ), dtype=np.int64)
    np.add.at(counts, (core, blk, q), 1)
    slots = np.maximum(1, _cdiv_np(counts.max(axis=0), P))  # [NB, NQ] shared

    kb = slots.sum(axis=1)  # slots per block across chunks
    K_total = int(kb.sum())
    IW = K_total * 8  # int16 idx columns (128 idxs -> 8 cols of 16)

    # tn layout: per block b, [128, 2*kb[b]] at tn_off[b]
    tn_off = np.zeros(NB, dtype=np.int64)
    tn_off[1:] = np.cumsum(2 * kb)[:-1]
    gslot_off = np.zeros((NB, NQ), dtype=np.int64)  # within-block slot offset
    gslot_off[:, 1:] = np.cumsum(slots, axis=1)[:, :-1]

    # gather-call layout, (g, q) calls; within a call blocks are contiguous
    slots_gq = np.zeros((NG, NQ), dtype=np.int64)
    sec_off = np.zeros((NB, NQ), dtype=np.int64)  # position offset within call
    for g in range(NG):
        bs = range(g * G, min((g + 1) * G, NB))
        acc = np.zeros(NQ, dtype=np.int64)
        for b in bs:
            sec_off[b, :] = acc * P
            acc += slots[b, :]
        slots_gq[g, :] = acc

    ix_off = np.zeros((NG, NQ), dtype=np.int64)  # idx16 column offset per call
    glob_off = np.zeros((NG, NQ), dtype=np.int64)  # position offset per call
    acc = 0
    for g in range(NG):
        for qq in range(NQ):
            ix_off[g, qq] = acc // 16
            glob_off[g, qq] = acc
            acc += int(slots_gq[g, qq]) * P
    assert acc == K_total * P
    assert acc // 16 == IW

    order = np.lexsort((src, blk, q, grp, core))
    so_src_local = sloc[order]
    so_norm = norm[order]
    so_tl = tloc[order]
    so_core = core[order]
    so_blk = blk[order]
    so_q = q[order]
    so_grp = grp[order]

    per_core = []
    for c in range(NC):
        m = so_core == c
        cb, cq, cg = so_blk[m], so_q[m], so_grp[m]
        csl, cn, ctl = so_src_local[m], so_norm[m], so_tl[m]
        gid = cb * NQ + cq
        pos = _group_cumcount(gid)  # position within (b, q)

        idx_flat = np.zeros(K_total * P, dtype=np.int16)  # dummies gather row 0
        gp = glob_off[cg, cq] + sec_off[cb, cq] + pos
        idx_flat[gp] = csl.astype(np.int16)
        idx16 = np.ascontiguousarray(idx_flat.reshape(-1, 16).T)  # [16, IW]

        tn = np.zeros((P, 2 * K_total), dtype=np.float16)
        for b in range(NB):
            o = int(tn_off[b])
            tn[:, o : o + int(kb[b])] = -1.0
        prow = pos % P
        pcol = tn_off[cb] + gslot_off[cb, cq] + pos // P
        tn[prow, pcol] = ctl.astype(np.float16)
        tn[prow, pcol + kb[cb]] = cn.astype(np.float16)

        per_core.append({"idx16": np.tile(idx16, (8, 1)), "tn": tn})

    return {
        "slots": slots,
        "slots_gq": slots_gq,
        "sec_off": sec_off,
        "kb": kb,
        "K_total": K_total,
        "IW": IW,
        "tn_off": tn_off,
        "ix_off": ix_off,
        "gslot_off": gslot_off,
        "NB": NB,
        "NG": NG,
        "NQ": NQ,
        "per_core": per_core,
    }


def _build_program(meta, n_nodes: int, chunk: int, fin, fh, fout):
    N = n_nodes
    S = N // NC
    NB = meta["NB"]
    NG = meta["NG"]
    NQ = meta["NQ"]
    slots = meta["slots"]
    slots_gq = meta["slots_gq"]
    sec_off = meta["sec_off"]
    kb = meta["kb"]
    K_total = meta["K_total"]
    IW = meta["IW"]
    tn_off = meta["tn_off"]
    ix_off = meta["ix_off"]
    gslot_off = meta["gslot_off"]
    fo_pad = 128  # t3 row padded to 128 fp16 = 256B (gather granularity)

    nc = bacc.Bacc()

    xT = nc.dram_tensor("xT", [fin, S], F16, kind="ExternalInput")
    W1 = nc.dram_tensor("W1", [fin, fh], F16, kind="ExternalInput")
    W2 = nc.dram_tensor("W2", [fh, fh], F16, kind="ExternalInput")
    W3 = nc.dram_tensor("W3", [fh, fout], F16, kind="ExternalInput")
    LW = nc.dram_tensor("LW", [2 * fh + fout, fout], F16, kind="ExternalInput")
    idx16 = nc.dram_tensor("idx16", [P, IW], I16, kind="ExternalInput")
    tn = nc.dram_tensor("tn", [P, 2 * K_total], F16, kind="ExternalInput")
    iota_in = nc.dram_tensor("iota", [P, P], F16, kind="ExternalInput")
    ident_in = nc.dram_tensor("ident", [P, P], F16, kind="ExternalInput")
    b1bc_in = nc.dram_tensor("b1bc", [P, fh], F16, kind="ExternalInput")
    b2bc_in = nc.dram_tensor("b2bc", [P, fh], F16, kind="ExternalInput")
    b3bc_in = nc.dram_tensor("b3bc", [P, fout], F16, kind="ExternalInput")
    lbbc_in = nc.dram_tensor("lbbc", [P, fout], F16, kind="ExternalInput")
    out_sh = nc.dram_tensor("out_sh", [S, fout], F32, kind="ExternalOutput")

    t1_sh = nc.dram_tensor("t1_sh", [S, fh], F16)
    t2_sh = nc.dram_tensor("t2_sh", [S, fh], F16)
    t3_sh = nc.dram_tensor("t3_sh", [S, fo_pad], F16)
    t1_full = nc.dram_tensor("t1_full", [N, fh], F16, addr_space="Shared")
    t2_full = nc.dram_tensor("t2_full", [N, fh], F16, addr_space="Shared")
    t3_full = nc.dram_tensor("t3_full", [N, fo_pad], F16, addr_space="Shared")
    h1T_sh = nc.dram_tensor("h1T_sh", [fh, S], F16)
    h2T_sh = nc.dram_tensor("h2T_sh", [fh, S], F16)

    rg = [list(range(NC))]

    def used_rows(b):
        return min(P, S - b * P)

    with tile.TileContext(nc) as tc:
        with (
            tc.tile_pool(name="const", bufs=1) as cpool,
            tc.tile_pool(name="sb", bufs=3) as pool,
            tc.tile_pool(name="gath", bufs=2) as gpool,
            tc.tile_pool(name="ps", bufs=1, space="PSUM") as psp,
            tc.tile_pool(name="ps2p", bufs=2, space="PSUM") as psp2,
        ):
            iota_t = cpool.tile([P, P], F16)
            nc.sync.dma_start(out=iota_t[:], in_=iota_in[:, :])
            ident_t = cpool.tile([P, P], F16)
            nc.sync.dma_start(out=ident_t[:], in_=ident_in[:, :])
            w1_t = cpool.tile([P, 2, fh], F16)
            nc.sync.dma_start(out=w1_t[:], in_=W1[:, :].rearrange("(c k) f -> k c f", k=P))
            w2_t = cpool.tile([P, 2, fh], F16)
            nc.sync.dma_start(out=w2_t[:], in_=W2[:, :].rearrange("(c k) f -> k c f", k=P))
            w3_t = cpool.tile([P, 2, fout], F16)
            nc.sync.dma_start(out=w3_t[:], in_=W3[:, :].rearrange("(c k) f -> k c f", k=P))
            lw12_t = cpool.tile([P, 4, fout], F16)
            nc.sync.dma_start(
                out=lw12_t[:], in_=LW[: 4 * P, :].rearrange("(c k) f -> k c f", k=P)
            )
            lw3_t = cpool.tile([fout, fout], F16)
            nc.sync.dma_start(out=lw3_t[:], in_=LW[4 * P :, :])
            b1bc = cpool.tile([P, fh], F16)
            nc.sync.dma_start(out=b1bc[:], in_=b1bc_in[:, :])
            b2bc = cpool.tile([P, fh], F16)
            nc.sync.dma_start(out=b2bc[:], in_=b2bc_in[:, :])
            b3bc = cpool.tile([P, fout], F16)
            nc.sync.dma_start(out=b3bc[:], in_=b3bc_in[:, :])
            lbbc = cpool.tile([P, fout], F16)
            nc.sync.dma_start(out=lbbc[:], in_=lbbc_in[:, :])

            sc_T = nc.enter_named_scope("phaseT", False)
            for b in range(NB):
                u = used_rows(b)
                ps1 = psp2.tile([P, fh], F32, tag="ps2")
                for cc in range(2):
                    xt = pool.tile([P, P], F16, tag="xt")
                    nc.sync.dma_start(
                        out=xt[:, :u], in_=xT[cc * P : (cc + 1) * P, b * P : b * P + u]
                    )
                    nc.tensor.matmul(
                        ps1[:u, :],
                        lhsT=xt[:, :u],
                        rhs=w1_t[:, cc, :],
                        start=(cc == 0),
                        stop=(cc == 1),
                    )
                ev = pool.tile([P, fh], F16, tag="ev")
                nc.scalar.activation(ev[:u, :], ps1[:u, :], AF.Copy)
                nc.sync.dma_start(out=t1_sh[b * P : b * P + u, :], in_=ev[:u, :])

            nc.leave_named_scope("phaseT", sc_T[0], False)
            sc = nc.enter_named_scope("ag1", False)
            nc.gpsimd.collective_compute(
                "AllGather", AL.bypass, ins=[t1_sh[:, :]], outs=[t1_full[:, :]],
                replica_groups=rg,
            )
            nc.leave_named_scope("ag1", sc[0], False)

            def layer(li, table, felem, wnext_t, fnext, bias_bc, tnext_sh, hT_sh_):
                fagg = fh if li < 3 else fout
                for g in range(NG):
                    b_lo = g * G
                    b_hi = min((g + 1) * G, NB)
                    nb_g = b_hi - b_lo
                    tno = int(tn_off[b_lo])
                    tn_w = int(tn_off[b_hi - 1] + 2 * kb[b_hi - 1]) - tno
                    tnt = pool.tile([P, 2 * G * 24], F16, tag="tnt")
                    nc.sync.dma_start(out=tnt[:, :tn_w], in_=tn[:, tno : tno + tn_w])

                    psas = [
                        psp.tile([P, fh], F32, tag=f"psa{i}") for i in range(nb_g)
                    ]
                    first = [True] * nb_g
                    for qq in range(NQ):
                        sl_gq = int(slots_gq[g, qq])
                        nidx = sl_gq * P
                        ixt = pool.tile([P, 46 * 8], I16, tag="ixt")
                        io = int(ix_off[g, qq])
                        nc.sync.dma_start(
                            out=ixt[:, : sl_gq * 8], in_=idx16[:, io : io + sl_gq * 8]
                        )
                        dst = gpool.tile([P, 46, felem], F16, tag="dst")
                        base = qq * chunk
                        rows = min(chunk, N - base)
                        nc.gpsimd.dma_gather(
                            dst[:, :sl_gq, :],
                            table[base : base + rows, :],
                            ixt[:, : sl_gq * 8],
                            nidx,
                            nidx,
                            felem,
                            single_packet=False,
                        )
                        for bi in range(nb_g):
                            b = b_lo + bi
                            col0 = int(sec_off[b, qq]) // P
                            tb = tno_rel = int(tn_off[b]) - tno
                            kbb = int(kb[b])
                            for j in range(int(slots[b, qq])):
                                s = int(gslot_off[b, qq]) + j
                                st = pool.tile([P, P], F16, tag="st")
                                nc.vector.tensor_scalar(
                                    out=st[:],
                                    in0=iota_t[:],
                                    scalar1=tnt[:, tb + s : tb + s + 1],
                                    scalar2=tnt[:, tb + kbb + s : tb + kbb + s + 1],
                                    op0=AL.is_equal,
                                    op1=AL.mult,
                                )
                                nc.tensor.matmul(
                                    psas[bi][:, :fagg],
                                    lhsT=st[:],
                                    rhs=dst[:, col0 + j, :fagg],
                                    start=first[bi],
                                    stop=False,
                                )
                                first[bi] = False

                    for bi in range(nb_g):
                        b = b_lo + bi
                        u = used_rows(b)
                        psa = psas[bi]
                        nc.tensor.matmul(
                            psa[:, :fagg], lhsT=ident_t[:], rhs=bias_bc[:, :fagg],
                            start=False, stop=True,
                        )
                        h_sb = pool.tile([P, fagg], F16, tag="h_sb")
                        nc.scalar.activation(h_sb[:], psa[:, :fagg], AF.Relu)

                        if li < 3:
                            ps2 = psp2.tile([P, fnext], F32, tag="ps2")
                            for cc in range(2):
                                pst = psp2.tile([P, P], F32, tag=f"pst{cc}")
                                nc.tensor.transpose(
                                    pst[:], h_sb[:, cc * P : (cc + 1) * P], ident_t[:]
                                )
                                hT = pool.tile([P, P], F16, tag=f"hT{cc}")
                                nc.vector.tensor_copy(hT[:], pst[:])
                                nc.sync.dma_start(
                                    out=hT_sh_[cc * P : (cc + 1) * P, b * P : b * P + u],
                                    in_=hT[:, :u],
                                )
                                nc.tensor.matmul(
                                    ps2[:u, :],
                                    lhsT=hT[:, :u],
                                    rhs=wnext_t[:, cc, :fnext],
                                    start=(cc == 0),
                                    stop=(cc == 1),
                                )
                            ev2 = pool.tile([P, fnext], F16, tag="ev")
                            nc.scalar.activation(ev2[:u, :fnext], ps2[:u, :], AF.Copy)
                            nc.sync.dma_start(
                                out=tnext_sh[b * P : b * P + u, :fnext],
                                in_=ev2[:u, :fnext],
                            )
                        else:
                            ps3t = psp2.tile([fout, P], F32, tag="pst0")
                            nc.tensor.transpose(ps3t[:], h_sb[:, :fout], ident_t[:])
                            h3T = pool.tile([fout, P], F16, tag="hT0")
                            nc.vector.tensor_copy(h3T[:], ps3t[:])
                            pso = psp2.tile([P, fout], F32, tag="ps2")
                            for cc in range(2):
                                r1 = pool.tile([P, P], F16, tag=f"rl{cc}")
                                nc.sync.dma_start(
                                    out=r1[:, :u],
                                    in_=h1T_sh[cc * P : (cc + 1) * P, b * P : b * P + u],
                                )
                                nc.tensor.matmul(
                                    pso[:u, :], lhsT=r1[:, :u], rhs=lw12_t[:, cc, :],
                                    start=(cc == 0), stop=False,
                                )
                            for cc in range(2):
                                r2 = pool.tile([P, P], F16, tag=f"rl{2 + cc}")
                                nc.sync.dma_start(
                                    out=r2[:, :u],
                                    in_=h2T_sh[cc * P : (cc + 1) * P, b * P : b * P + u],
                                )
                                nc.tensor.matmul(
                                    pso[:u, :], lhsT=r2[:, :u], rhs=lw12_t[:, 2 + cc, :],
                                    start=False, stop=False,
                                )
                            nc.tensor.matmul(
                                pso[:u, :], lhsT=h3T[:, :u], rhs=lw3_t[:, :],
                                start=False, stop=False,
                            )
                            nc.tensor.matmul(
                                pso[:u, :], lhsT=ident_t[:, :u], rhs=lbbc[:, :],
                                start=False, stop=True,
                            )
                            m_t = pool.tile([P, 1], F32, tag="m_t")
                            nc.vector.tensor_reduce(
                                m_t[:u, :], pso[:u, :], mybir.AxisListType.X, AL.max
                            )
                            nm_t = pool.tile([P, 1], F32, tag="nm_t")
                            nc.vector.tensor_scalar(
                                out=nm_t[:u, :], in0=m_t[:u, :], scalar1=-1.0,
                                scalar2=None, op0=AL.mult,
                            )
                            e_t = pool.tile([P, fout], F32, tag="e_t")
                            ssum = pool.tile([P, 1], F32, tag="ssum")
                            nc.scalar.activation(
                                e_t[:u, :], pso[:u, :], AF.Exp,
                                bias=nm_t[:u, :1], accum_out=ssum[:u, :1],
                            )
                            ls_t = pool.tile([P, 1], F32, tag="ls_t")
                            nc.scalar.activation(ls_t[:u, :], ssum[:u, :], AF.Ln)
                            mls = pool.tile([P, 1], F32, tag="mls")
                            nc.vector.tensor_tensor(
                                out=mls[:u, :], in0=m_t[:u, :], in1=ls_t[:u, :],
                                op=AL.add,
                            )
                            z_t = pool.tile([P, fout], F32, tag="z_t")
                            nc.vector.tensor_scalar(
                                out=z_t[:u, :], in0=pso[:u, :], scalar1=mls[:u, :1],
                                scalar2=None, op0=AL.subtract,
                            )
                            nc.sync.dma_start(
                                out=out_sh[b * P : b * P + u, :], in_=z_t[:u, :]
                            )

            sc = nc.enter_named_scope("L1", False)
            layer(1, t1_full, fh, w2_t, fh, b1bc, t2_sh, h1T_sh)
            nc.leave_named_scope("L1", sc[0], False)
            sc = nc.enter_named_scope("ag2", False)
            nc.gpsimd.collective_compute(
                "AllGather", AL.bypass, ins=[t2_sh[:, :]], outs=[t2_full[:, :]],
                replica_groups=rg,
            )
            nc.leave_named_scope("ag2", sc[0], False)
            sc = nc.enter_named_scope("L2", False)
            layer(2, t2_full, fh, w3_t, fout, b2bc, t3_sh, h2T_sh)
            nc.leave_named_scope("L2", sc[0], False)
            sc = nc.enter_named_scope("ag3", False)
            nc.gpsimd.collective_compute(
                "AllGather", AL.bypass, ins=[t3_sh[:, :]], outs=[t3_full[:, :]],
                replica_groups=rg,
            )
            nc.leave_named_scope("ag3", sc[0], False)
            sc = nc.enter_named_scope("L3", False)
            layer(3, t3_full, fo_pad, None, None, b3bc, None, None)
            nc.leave_named_scope("L3", sc[0], False)

    nc.finalize()
    return nc


def _prepare(x, edge_index, W1, b1, W2, b2, W3, b3, lin_w, lin_b, chunk):
    x = np.asarray(x)
    N = x.shape[0]
    S = N // NC
    fin, fh, fout = W1.shape[0], W2.shape[0], W3.shape[1]

    meta = _preprocess(np.asarray(edge_index, dtype=np.int64), N, chunk)
    nc = _build_program(meta, N, chunk, fin, fh, fout)

    iota = np.tile(np.arange(P, dtype=np.float16), (P, 1))
    ident = np.eye(P, dtype=np.float16)
    b1bc = np.tile(np.asarray(b1, np.float16), (P, 1))
    b2bc = np.tile(np.asarray(b2, np.float16), (P, 1))
    b3bc = np.tile(np.asarray(b3, np.float16), (P, 1))
    lbbc = np.tile(np.asarray(lin_b, np.float16), (P, 1))

    in_maps = []
    for c in range(NC):
        xs = np.asarray(x[c * S : (c + 1) * S], np.float16)
        in_maps.append(
            {
                "xT": np.ascontiguousarray(xs.T),
                "W1": np.asarray(W1, np.float16),
                "W2": np.asarray(W2, np.float16),
                "W3": np.asarray(W3, np.float16),
                "LW": np.asarray(lin_w, np.float16),
                "idx16": meta["per_core"][c]["idx16"],
                "tn": meta["per_core"][c]["tn"],
                "iota": iota,
                "ident": ident,
                "b1bc": b1bc,
                "b2bc": b2bc,
                "b3bc": b3bc,
                "lbbc": lbbc,
            }
        )
    return nc, in_maps


def kernel(x, edge_index, W1, b1, W2, b2, W3, b3, lin_w, lin_b):
    global LAST_EXEC_NS, LAST_SCOPES
    nc, in_maps = _prepare(
        x, edge_index, W1, b1, W2, b2, W3, b3, lin_w, lin_b, chunk=25000
    )
    trace = bool(os.environ.get("GCN_TRACE"))
    res = run_bass_kernel_spmd(nc, in_maps, list(range(NC)), trace=trace)
    LAST_EXEC_NS = res.exec_time_ns
    LAST_SCOPES = res.per_core_scope_times
    S = np.asarray(x).shape[0] // NC
    out = np.concatenate([res.results[c]["out_sh"] for c in range(NC)], axis=0)
    return out.astype(np.float32)


# revision 17
# speedup vs baseline: 1.0317x; 1.0317x over previous
"""GCN 3-layer message-passing kernel for TRN2 (8 NeuronCores, SPMD).

Self-contained: takes FULL inputs, shards internally, runs a Bass/Tile
kernel via run_bass_kernel_spmd on cores 0-7, gathers the full output.

v2 strategy (vs baseline):
  - fp16 tables everywhere: halves gather DMA, AllGather bytes, and
    doubles PE matmul throughput. Validated ~3e-5 rel err on host.
  - Gathers grouped over G=8 target blocks per source chunk: 52
    dma_gather calls per layer instead of 392, amortizing the ~5us
    per-call SWDGE cost on the Pool engine (measured on HW).
  - Slot-aligned equalized padding: every (block, chunk) section is
    padded with dummy index-0 entries to its shared slot count, so a
    grouped gather call has no interior negative indices and num_idxs
    is a compile-time immediate (no count registers at all).
  - Batched one-hot builds: per gather call, ALL slot one-hot matrices
    are built with 2 DVE tensor_tensor ops (is_equal vs a column-mod
    iota, then mult by norm) using stride-0 broadcast APs — instead of
    one ~0.9us tensor_scalar per slot (measured on HW).
  - Per-group PSUM accumulation across the 4 source chunks; bias via
    identity matmul; relu fused in the ACT evacuation.
"""

import os
import sys

sys.path.insert(0, "/opt/trn_rl_repo")

import numpy as np

import concourse.bass as bass  # noqa: F401
import concourse.mybir as mybir
import concourse.tile as tile
from concourse import bacc
from concourse._compat import cdiv
from concourse.bass_utils import run_bass_kernel_spmd

F32 = mybir.dt.float32
F16 = mybir.dt.float16
I16 = mybir.dt.int16
AL = mybir.AluOpType
AF = mybir.ActivationFunctionType

NC = 8
P = 128
G = 8  # target blocks per gather group

LAST_EXEC_NS = None
LAST_SCOPES = None


def _cdiv_np(a, b):
    return (a + b - 1) // b


def _group_cumcount(grp: np.ndarray) -> np.ndarray:
    n = len(grp)
    if n == 0:
        return np.zeros(0, dtype=np.int64)
    is_new = np.ones(n, dtype=bool)
    is_new[1:] = grp[1:] != grp[:-1]
    idx = np.arange(n)
    start = np.maximum.accumulate(np.where(is_new, idx, 0))
    return idx - start


def _preprocess(edge_index: np.ndarray, n_nodes: int, chunk: int):
    N = n_nodes
    S = N // NC
    NB = cdiv(S, P)
    NG = _cdiv_np(NB, G)
    NQ = cdiv(N, chunk)

    src = np.concatenate([edge_index[0], np.arange(N, dtype=np.int64)])
    dst = np.concatenate([edge_index[1], np.arange(N, dtype=np.int64)])
    deg = np.bincount(dst, minlength=N).astype(np.float64)
    dis = 1.0 / np.sqrt(deg)
    norm = (dis[src] * dis[dst]).astype(np.float32)

    core = dst // S
    blk = (dst % S) // P
    tloc = (dst % S) % P
    grp = blk // G
    q = src // chunk
    sloc = src - q * chunk

    counts = np.zeros((NC, NB, NQ), dtype=np.int64)
    np.add.at(counts, (core, blk, q), 1)
    slots = np.maximum(1, _cdiv_np(counts.max(axis=0), P))  # [NB, NQ] shared

    kb = slots.sum(axis=1)  # slots per block across chunks
    K_total = int(kb.sum())
    IW = K_total * 8  # int16 idx columns (128 idxs -> 8 cols of 16)

    # gather-call layout, (g, q) calls; within a call blocks are contiguous
    slots_gq = np.zeros((NG, NQ), dtype=np.int64)
    sec_off = np.zeros((NB, NQ), dtype=np.int64)  # position offset within call
    for g in range(NG):
        bs = range(g * G, min((g + 1) * G, NB))
        acc = np.zeros(NQ, dtype=np.int64)
        for b in bs:
            sec_off[b, :] = acc * P
            acc += slots[b, :]
        slots_gq[g, :] = acc
    MAXSL = int(slots_gq.max())

    ix_off = np.zeros((NG, NQ), dtype=np.int64)  # idx16 column offset per call
    glob_off = np.zeros((NG, NQ), dtype=np.int64)  # position offset per call
    tn2_off = np.zeros((NG, NQ), dtype=np.int64)  # tn column offset per call
    acc = 0
    tacc = 0
    for g in range(NG):
        for qq in range(NQ):
            ix_off[g, qq] = acc // 16
            glob_off[g, qq] = acc
            tn2_off[g, qq] = tacc
            acc += int(slots_gq[g, qq]) * P
            tacc += 2 * int(slots_gq[g, qq])
    assert acc == K_total * P
    assert acc // 16 == IW
    assert tacc == 2 * K_total

    order = np.lexsort((src, blk, q, grp, core))
    so_src_local = sloc[order]
    so_norm = norm[order]
    so_tl = tloc[order]
    so_core = core[order]
    so_blk = blk[order]
    so_q = q[order]
    so_grp = grp[order]

    per_core = []
    for c in range(NC):
        m = so_core == c
        cb, cq, cg = so_blk[m], so_q[m], so_grp[m]
        csl, cn, ctl = so_src_local[m], so_norm[m], so_tl[m]
        gid = cb * NQ + cq
        pos = _group_cumcount(gid)  # position within (b, q)

        idx_flat = np.zeros(K_total * P, dtype=np.int16)  # dummies gather row 0
        gp = glob_off[cg, cq] + sec_off[cb, cq] + pos
        idx_flat[gp] = csl.astype(np.int16)
        idx16 = np.ascontiguousarray(idx_flat.reshape(-1, 16).T)  # [16, IW]

        # call-major tn: per (g, q) region [tv cols | norm cols], each
        # slots_gq wide; dummy slots stay (tv=-1, norm=0)
        tn = np.zeros((P, 2 * K_total), dtype=np.float16)
        for g in range(NG):
            for qq in range(NQ):
                o = int(tn2_off[g, qq])
                tn[:, o : o + int(slots_gq[g, qq])] = -1.0
        prow = pos % P
        call_slot = (sec_off[cb, cq] + pos) // P  # slot within the call
        pcol = tn2_off[cg, cq] + call_slot
        tn[prow, pcol] = ctl.astype(np.float16)
        tn[prow, pcol + slots_gq[cg, cq]] = cn.astype(np.float16)

        per_core.append({"idx16": np.tile(idx16, (8, 1)), "tn": tn})

    return {
        "slots": slots,
        "slots_gq": slots_gq,
        "sec_off": sec_off,
        "kb": kb,
        "K_total": K_total,
        "IW": IW,
        "MAXSL": MAXSL,
        "ix_off": ix_off,
        "tn2_off": tn2_off,
        "NB": NB,
        "NG": NG,
        "NQ": NQ,
        "per_core": per_core,
    }


def _build_program(meta, n_nodes: int, chunk: int, fin, fh, fout):
    N = n_nodes
    S = N // NC
    NB = meta["NB"]
    NG = meta["NG"]
    NQ = meta["NQ"]
    slots = meta["slots"]
    slots_gq = meta["slots_gq"]
    sec_off = meta["sec_off"]
    K_total = meta["K_total"]
    IW = meta["IW"]
    MAXSL = meta["MAXSL"]
    ix_off = meta["ix_off"]
    tn2_off = meta["tn2_off"]
    fo_pad = 128  # t3 row padded to 128 fp16 = 256B (gather granularity)

    nc = bacc.Bacc()

    xT = nc.dram_tensor("xT", [fin, S], F16, kind="ExternalInput")
    W1 = nc.dram_tensor("W1", [fin, fh], F16, kind="ExternalInput")
    W2 = nc.dram_tensor("W2", [fh, fh], F16, kind="ExternalInput")
    W3 = nc.dram_tensor("W3", [fh, fout], F16, kind="ExternalInput")
    LW = nc.dram_tensor("LW", [2 * fh + fout, fout], F16, kind="ExternalInput")
    idx16 = nc.dram_tensor("idx16", [P, IW], I16, kind="ExternalInput")
    tn = nc.dram_tensor("tn", [P, 2 * K_total], F16, kind="ExternalInput")
    iotam_in = nc.dram_tensor("iotam", [P, MAXSL * P], F16, kind="ExternalInput")
    ident_in = nc.dram_tensor("ident", [P, P], F16, kind="ExternalInput")
    b1bc_in = nc.dram_tensor("b1bc", [P, fh], F16, kind="ExternalInput")
    b2bc_in = nc.dram_tensor("b2bc", [P, fh], F16, kind="ExternalInput")
    b3bc_in = nc.dram_tensor("b3bc", [P, fout], F16, kind="ExternalInput")
    lbbc_in = nc.dram_tensor("lbbc", [P, fout], F16, kind="ExternalInput")
    out_sh = nc.dram_tensor("out_sh", [S, fout], F32, kind="ExternalOutput")

    t1_sh = nc.dram_tensor("t1_sh", [S, fh], F16)
    t2_sh = nc.dram_tensor("t2_sh", [S, fh], F16)
    t3_sh = nc.dram_tensor("t3_sh", [S, fo_pad], F16)
    t1_full = nc.dram_tensor("t1_full", [N, fh], F16, addr_space="Shared")
    t2_full = nc.dram_tensor("t2_full", [N, fh], F16, addr_space="Shared")
    t3_full = nc.dram_tensor("t3_full", [N, fo_pad], F16, addr_space="Shared")
    h1T_sh = nc.dram_tensor("h1T_sh", [fh, S], F16)
    h2T_sh = nc.dram_tensor("h2T_sh", [fh, S], F16)

    rg = [list(range(NC))]

    def used_rows(b):
        return min(P, S - b * P)

    with tile.TileContext(nc) as tc:
        with (
            tc.tile_pool(name="const", bufs=1) as cpool,
            tc.tile_pool(name="sb", bufs=3) as pool,
            tc.tile_pool(name="gath", bufs=2) as gpool,
            tc.tile_pool(name="ps", bufs=1, space="PSUM") as psp,
            tc.tile_pool(name="ps2p", bufs=2, space="PSUM") as psp2,
        ):
            iotam_t = cpool.tile([P, MAXSL * P], F16)
            nc.sync.dma_start(out=iotam_t[:], in_=iotam_in[:, :])
            ident_t = cpool.tile([P, P], F16)
            nc.sync.dma_start(out=ident_t[:], in_=ident_in[:, :])
            w1_t = cpool.tile([P, 2, fh], F16)
            nc.sync.dma_start(out=w1_t[:], in_=W1[:, :].rearrange("(c k) f -> k c f", k=P))
            w2_t = cpool.tile([P, 2, fh], F16)
            nc.sync.dma_start(out=w2_t[:], in_=W2[:, :].rearrange("(c k) f -> k c f", k=P))
            w3_t = cpool.tile([P, 2, fout], F16)
            nc.sync.dma_start(out=w3_t[:], in_=W3[:, :].rearrange("(c k) f -> k c f", k=P))
            lw12_t = cpool.tile([P, 4, fout], F16)
            nc.sync.dma_start(
                out=lw12_t[:], in_=LW[: 4 * P, :].rearrange("(c k) f -> k c f", k=P)
            )
            lw3_t = cpool.tile([fout, fout], F16)
            nc.sync.dma_start(out=lw3_t[:], in_=LW[4 * P :, :])
            b1bc = cpool.tile([P, fh], F16)
            nc.sync.dma_start(out=b1bc[:], in_=b1bc_in[:, :])
            b2bc = cpool.tile([P, fh], F16)
            nc.sync.dma_start(out=b2bc[:], in_=b2bc_in[:, :])
            b3bc = cpool.tile([P, fout], F16)
            nc.sync.dma_start(out=b3bc[:], in_=b3bc_in[:, :])
            lbbc = cpool.tile([P, fout], F16)
            nc.sync.dma_start(out=lbbc[:], in_=lbbc_in[:, :])
            zcon = cpool.tile([P, 2 * fh], F16)
            nc.vector.memset(zcon[:], 0.0)

            sc_T = nc.enter_named_scope("phaseT", False)
            for b in range(NB):
                u = used_rows(b)
                ps1 = psp2.tile([P, fh], F32, tag="ps2")
                for cc in range(2):
                    xt = pool.tile([P, P], F16, tag="xt")
                    nc.sync.dma_start(
                        out=xt[:, :u], in_=xT[cc * P : (cc + 1) * P, b * P : b * P + u]
                    )
                    nc.tensor.matmul(
                        ps1[:u, :],
                        lhsT=xt[:, :u],
                        rhs=w1_t[:, cc, :],
                        start=(cc == 0),
                        stop=(cc == 1),
                    )
                ev = pool.tile([P, fh], F16, tag="ev")
                nc.scalar.activation(ev[:u, :], ps1[:u, :], AF.Copy)
                nc.sync.dma_start(out=t1_sh[b * P : b * P + u, :], in_=ev[:u, :])

            nc.leave_named_scope("phaseT", sc_T[0], False)
            sc = nc.enter_named_scope("ag1", False)
            nc.gpsimd.collective_compute(
                "AllGather", AL.bypass, ins=[t1_sh[:, :]], outs=[t1_full[:, :]],
                replica_groups=rg,
            )
            nc.leave_named_scope("ag1", sc[0], False)

            def layer(li, table, felem, wnext_t, fnext, bias_bc, tnext_sh, hT_sh_,
                      fnext_pad=None):
                fagg = fh if li < 3 else fout
                for g in range(NG):
                    b_lo = g * G
                    b_hi = min((g + 1) * G, NB)
                    nb_g = b_hi - b_lo

                    psabs = [
                        psp.tile([P, 2 * fh], F32, tag=f"psab{i}", name=f"psab{i}")
                        for i in range((nb_g + 1) // 2)
                    ]
                    psas = [
                        psabs[i // 2][:, (i % 2) * fh : (i % 2 + 1) * fh]
                        for i in range(nb_g)
                    ]
                    for t in psabs:
                        nc.tensor.matmul(
                            t[:], lhsT=ident_t[:], rhs=zcon[:],
                            start=True, stop=False,
                        )
                    for qq in range(NQ):
                        sl_gq = int(slots_gq[g, qq])
                        nidx = sl_gq * P
                        ixt = pool.tile([P, MAXSL * 8], I16, tag="ixt")
                        io = int(ix_off[g, qq])
                        nc.sync.dma_start(
                            out=ixt[:, : sl_gq * 8], in_=idx16[:, io : io + sl_gq * 8]
                        )
                        tnc = pool.tile([P, 2 * MAXSL], F16, tag="tnc")
                        tno = int(tn2_off[g, qq])
                        nc.sync.dma_start(
                            out=tnc[:, : 2 * sl_gq], in_=tn[:, tno : tno + 2 * sl_gq]
                        )
                        dst = gpool.tile([P, MAXSL, felem], F16, tag="dst")
                        base = qq * chunk
                        rows = min(chunk, N - base)
                        nc.gpsimd.dma_gather(
                            dst[:, :sl_gq, :],
                            table[base : base + rows, :],
                            ixt[:, : sl_gq * 8],
                            nidx,
                            nidx,
                            felem,
                            single_packet=False,
                        )
                        # batched one-hot: all sl_gq slot matrices in 2 DVE ops
                        stb = gpool.tile([P, MAXSL, P], F16, tag="stb")
                        tv_b = tnc[:, 0:sl_gq, None].broadcast_to([P, sl_gq, P])
                        nm_b = tnc[:, sl_gq : 2 * sl_gq, None].broadcast_to(
                            [P, sl_gq, P]
                        )
                        nc.vector.tensor_tensor(
                            out=stb[:, :sl_gq, :],
                            in0=iotam_t[:, : sl_gq * P].rearrange(
                                "p (s c) -> p s c", c=P
                            ),
                            in1=tv_b,
                            op=AL.is_equal,
                        )
                        nc.vector.tensor_tensor(
                            out=stb[:, :sl_gq, :],
                            in0=stb[:, :sl_gq, :],
                            in1=nm_b,
                            op=AL.mult,
                        )
                        for bi in range(nb_g):
                            b = b_lo + bi
                            col0 = int(sec_off[b, qq]) // P
                            for j in range(int(slots[b, qq])):
                                s = col0 + j
                                nc.tensor.matmul(
                                    psas[bi][:, :fagg],
                                    lhsT=stb[:, s, :],
                                    rhs=dst[:, s, :fagg],
                                    start=False,
                                    stop=False,
                                )

                    for bi in range(nb_g):
                        tile_last = (bi % 2 == 1) or (bi == nb_g - 1)
                        nc.tensor.matmul(
                            psas[bi][:, :fagg],
                            lhsT=ident_t[:], rhs=bias_bc[:, :fagg],
                            start=False, stop=tile_last,
                        )
                    for bi in range(nb_g):
                        b = b_lo + bi
                        u = used_rows(b)
                        psa = psas[bi]
                        h_sb = pool.tile([P, fagg], F16, tag="h_sb")
                        nc.scalar.activation(h_sb[:], psa[:, :fagg], AF.Relu)

                        if li < 3:
                            ps2 = psp2.tile([P, fnext], F32, tag="ps2")
                            for cc in range(2):
                                pst = psp2.tile([P, P], F16, tag="pst")
                                nc.tensor.transpose(
                                    pst[:], h_sb[:, cc * P : (cc + 1) * P], ident_t[:]
                                )
                                hT = pool.tile([P, P], F16, tag="hT")
                                nc.vector.tensor_copy(hT[:], pst[:])
                                nc.sync.dma_start(
                                    out=hT_sh_[cc * P : (cc + 1) * P, b * P : b * P + u],
                                    in_=hT[:, :u],
                                )
                                nc.tensor.matmul(
                                    ps2[:u, :],
                                    lhsT=hT[:, :u],
                                    rhs=wnext_t[:, cc, :fnext],
                                    start=(cc == 0),
                                    stop=(cc == 1),
                                )
                            fp = fnext_pad or fnext
                            ev2 = pool.tile([P, fp], F16, tag="ev")
                            nc.scalar.activation(ev2[:u, :fnext], ps2[:u, :], AF.Copy)
                            if fp > fnext:
                                nc.vector.memset(ev2[:u, fnext:fp], 0.0)
                            nc.sync.dma_start(
                                out=tnext_sh[b * P : b * P + u, :fp],
                                in_=ev2[:u, :fp],
                            )
                        else:
                            ps3t = psp2.tile([fout, P], F16, tag="pst")
                            nc.tensor.transpose(ps3t[:], h_sb[:, :fout], ident_t[:])
                            h3T = pool.tile([fout, P], F16, tag="hT")
                            nc.vector.tensor_copy(h3T[:], ps3t[:])
                            pso = psp2.tile([P, fout], F32, tag="ps2")
                            for cc in range(2):
                                r1 = pool.tile([P, P], F16, tag=f"rl{cc}")
                                nc.sync.dma_start(
                                    out=r1[:, :u],
                                    in_=h1T_sh[cc * P : (cc + 1) * P, b * P : b * P + u],
                                )
                                nc.tensor.matmul(
                                    pso[:u, :], lhsT=r1[:, :u], rhs=lw12_t[:, cc, :],
                                    start=(cc == 0), stop=False,
                                )
                            for cc in range(2):
                                r2 = pool.tile([P, P], F16, tag=f"rl{2 + cc}")
                                nc.sync.dma_start(
                                    out=r2[:, :u],
                                    in_=h2T_sh[cc * P : (cc + 1) * P, b * P : b * P + u],
                                )
                                nc.tensor.matmul(
                                    pso[:u, :], lhsT=r2[:, :u], rhs=lw12_t[:, 2 + cc, :],
                                    start=False, stop=False,
                                )
                            nc.tensor.matmul(
                                pso[:u, :], lhsT=h3T[:, :u], rhs=lw3_t[:, :],
                                start=False, stop=False,
                            )
                            nc.tensor.matmul(
                                pso[:u, :], lhsT=ident_t[:, :u], rhs=lbbc[:, :],
                                start=False, stop=True,
                            )
                            m_t = pool.tile([P, 1], F32, tag="m_t")
                            nc.vector.tensor_reduce(
                                m_t[:u, :], pso[:u, :], mybir.AxisListType.X, AL.max
                            )
                            nm_t = pool.tile([P, 1], F32, tag="nm_t")
                            nc.vector.tensor_scalar(
                                out=nm_t[:u, :], in0=m_t[:u, :], scalar1=-1.0,
                                scalar2=None, op0=AL.mult,
                            )
                            e_t = pool.tile([P, fout], F32, tag="e_t")
                            ssum = pool.tile([P, 1], F32, tag="ssum")
                            nc.scalar.activation(
                                e_t[:u, :], pso[:u, :], AF.Exp,
                                bias=nm_t[:u, :1], accum_out=ssum[:u, :1],
                            )
                            ls_t = pool.tile([P, 1], F32, tag="ls_t")
                            nc.scalar.activation(ls_t[:u, :], ssum[:u, :], AF.Ln)
                            mls = pool.tile([P, 1], F32, tag="mls")
                            nc.vector.tensor_tensor(
                                out=mls[:u, :], in0=m_t[:u, :], in1=ls_t[:u, :],
                                op=AL.add,
                            )
                            z_t = pool.tile([P, fout], F32, tag="z_t")
                            nc.vector.tensor_scalar(
                                out=z_t[:u, :], in0=pso[:u, :], scalar1=mls[:u, :1],
                                scalar2=None, op0=AL.subtract,
                            )
                            nc.sync.dma_start(
                                out=out_sh[b * P : b * P + u, :], in_=z_t[:u, :]
                            )

            sc = nc.enter_named_scope("L1", False)
            layer(1, t1_full, fh, w2_t, fh, b1bc, t2_sh, h1T_sh)
            nc.leave_named_scope("L1", sc[0], False)
            sc = nc.enter_named_scope("ag2", False)
            nc.gpsimd.collective_compute(
                "AllGather", AL.bypass, ins=[t2_sh[:, :]], outs=[t2_full[:, :]],
                replica_groups=rg,
            )
            nc.leave_named_scope("ag2", sc[0], False)
            sc = nc.enter_named_scope("L2", False)
            layer(2, t2_full, fh, w3_t, fout, b2bc, t3_sh, h2T_sh,
                  fnext_pad=fo_pad)
            nc.leave_named_scope("L2", sc[0], False)
            sc = nc.enter_named_scope("ag3", False)
            nc.gpsimd.collective_compute(
                "AllGather", AL.bypass, ins=[t3_sh[:, :]], outs=[t3_full[:, :]],
                replica_groups=rg,
            )
            nc.leave_named_scope("ag3", sc[0], False)
            sc = nc.enter_named_scope("L3", False)
            layer(3, t3_full, fo_pad, None, None, b3bc, None, None)
            nc.leave_named_scope("L3", sc[0], False)

    nc.finalize()
    return nc


def _prepare(x, edge_index, W1, b1, W2, b2, W3, b3, lin_w, lin_b, chunk):
    x = np.asarray(x)
    N = x.shape[0]
    S = N // NC
    fin, fh, fout = W1.shape[0], W2.shape[0], W3.shape[1]

    meta = _preprocess(np.asarray(edge_index, dtype=np.int64), N, chunk)
    nc = _build_program(meta, N, chunk, fin, fh, fout)

    iotam = np.tile(
        np.tile(np.arange(P, dtype=np.float16), meta["MAXSL"]), (P, 1)
    )
    ident = np.eye(P, dtype=np.float16)
    b1bc = np.tile(np.asarray(b1, np.float16), (P, 1))
    b2bc = np.tile(np.asarray(b2, np.float16), (P, 1))
    b3bc = np.tile(np.asarray(b3, np.float16), (P, 1))
    lbbc = np.tile(np.asarray(lin_b, np.float16), (P, 1))

    in_maps = []
    for c in range(NC):
        xs = np.asarray(x[c * S : (c + 1) * S], np.float16)
        in_maps.append(
            {
                "xT": np.ascontiguousarray(xs.T),
                "W1": np.asarray(W1, np.float16),
                "W2": np.asarray(W2, np.float16),
                "W3": np.asarray(W3, np.float16),
                "LW": np.asarray(lin_w, np.float16),
                "idx16": meta["per_core"][c]["idx16"],
                "tn": meta["per_core"][c]["tn"],
                "iotam": iotam,
                "ident": ident,
                "b1bc": b1bc,
                "b2bc": b2bc,
                "b3bc": b3bc,
                "lbbc": lbbc,
            }
        )
    return nc, in_maps


def kernel(x, edge_index, W1, b1, W2, b2, W3, b3, lin_w, lin_b):
    global LAST_EXEC_NS, LAST_SCOPES
    nc, in_maps = _prepare(
        x, edge_index, W1, b1, W2, b2, W3, b3, lin_w, lin_b, chunk=25000
    )
    trace = bool(os.environ.get("GCN_TRACE"))
    res = run_bass_kernel_spmd(nc, in_maps, list(range(NC)), trace=trace)
    LAST_EXEC_NS = res.exec_time_ns
    LAST_SCOPES = res.per_core_scope_times
    S = np.asarray(x).shape[0] // NC
    out = np.concatenate([res.results[c]["out_sh"] for c in range(NC)], axis=0)
    return out.astype(np.float32)
